# revision 4
# baseline (speedup 1.0000x reference)
"""Trainium2 Bass kernel for nn_Conv2d_Downsample.

Pipeline: blur(depthwise 4x4 [1,3,3,1]^T[1,3,3,1]/64, pad 2) then 3x3/stride-2
conv (EqualizedLR scale 1/sqrt(fan_in)) + bias.

Decomposition on device (per core, data-parallel over batch, 2 images/core):
  - blur = three 2-tap box passes along W, then three along H (exact: the
    [1,1] convolved 3x gives [1,3,3,1]; the 1/64 norm is folded into W).
  - conv = 18 accumulating fp32r matmuls per [128co x 512spatial] PSUM tile
    (2 ci-tiles x 9 taps), channels on partitions.
  - ScalarE adds bias during PSUM->SBUF copy.
Engines: PE matmuls, DVE h3/v1/v2/v3 box passes, GPSIMD h1/h2 + memsets,
ACT bias copies, HWDGE DMA.
"""
import json
import os
import sys

import numpy as np

for _p in ("/opt/trn_rl_repo", "/root/.axon_site/_ro/trn_rl_repo"):
    if os.path.isdir(_p) and _p not in sys.path:
        sys.path.append(_p)

# ---------------------------------------------------------------- constants
N_FULL, C_IN, H, W = 16, 256, 128, 128
C_OUT, KCONV, DOWN = 512, 3, 2
N_CORES = 8
N_PC = N_FULL // N_CORES          # images per core
HP = WP = H + 4                   # zero-padded (pad=2 each side)
HB = WB = HP - 3                  # blurred size (129)
HO = WO = 64                      # output spatial
R = 16                            # strip rows (xpad coords)
NS = (HP + R - 1) // R            # 9 strips (last has 4 rows)
NSC = HO // 8                     # 8 conv strips (8 out rows each)
XBR = 17                          # xb strip rows (16 + 1 duplicated)

_CACHE: dict = {}


# ------------------------------------------------------------- birfix patch
def _fix_bir(bir):
    """walrus here caps sync waits at 1/instr (2 for EventSemaphore); split
    excess waits onto preceding single-wait Drains on the same engine."""
    ctr = 0
    for fn in bir.get("functions", []):
        for blk in fn.get("blocks", []):
            insts = blk.get("instructions")
            if not insts:
                continue
            out = []
            for inst in insts:
                si = inst.get("sync_info")
                waits = (si or {}).get("on_wait") or []
                cap = 2 if inst.get("opcode") == "EventSemaphore" else 1
                if len(waits) > cap:
                    extra, keep = waits[:-cap], waits[-cap:]
                    for w in extra:
                        ctr += 1
                        out.append({
                            "debug": inst.get("debug"), "engine": inst["engine"],
                            "ins": [], "is_reset_sema": False,
                            "name": f"I-wfix-{ctr}", "opcode": "Drain", "outs": [],
                            "sync_info": {"on_update": [], "on_wait": [w]},
                        })
                    si["on_wait"] = keep
                out.append(inst)
            blk["instructions"] = out
    return bir


def _install_birfix():
    import concourse.bass as bass
    if getattr(bass.Bass, "_birfix_installed", False):
        return
    orig = bass.Bass.to_json_bytes

    def to_json_bytes(self, *a, **k):
        return json.dumps(_fix_bir(json.loads(orig(self, *a, **k)))).encode()

    bass.Bass.to_json_bytes = to_json_bytes
    bass.Bass._birfix_installed = True


# ------------------------------------------------------------ module build
def _build_module():
    import concourse.bass as bass
    import concourse.tile as tile
    import concourse.mybir as mybir

    F32 = mybir.dt.float32
    F32R = mybir.dt.float32r
    AF = mybir.ActivationFunctionType

    nc = bass.Bass()
    x_d = nc.dram_tensor("x", [N_PC, C_IN, H, W], F32, kind="ExternalInput")
    w_d = nc.dram_tensor("w", [2, 128, 36, 128], F32, kind="ExternalInput")
    b_d = nc.dram_tensor("b", [128, 4], F32, kind="ExternalInput")
    y_d = nc.dram_tensor("y", [N_PC, C_OUT, HO, WO], F32, kind="ExternalOutput")

    with tile.TileContext(nc) as tc:
        with (
            tc.tile_pool(name="wpool", bufs=1) as wpool,
            tc.tile_pool(name="wstage", bufs=1) as wstage,
            tc.tile_pool(name="xin", bufs=3) as xin_p,
            tc.tile_pool(name="htmp", bufs=2) as htmp_p,
            tc.tile_pool(name="h3p", bufs=2) as h3_p,
            tc.tile_pool(name="vtp", bufs=2) as vt_p,
            tc.tile_pool(name="xbp", bufs=2) as xb_p,
            tc.tile_pool(name="outp", bufs=4) as out_p,
            tc.tile_pool(name="psum", bufs=8, space="PSUM") as psum_p,
        ):
            # ---- weights: DMA f32 chunks, round to f32r via DVE copy
            w = wpool.tile([128, 72, 128], F32R)
            bias = wpool.tile([128, 4], F32)
            nc.sync.dma_start(bias[:], b_d[:])
            for ci_t in range(2):
                for c in range(6):
                    st = wstage.tile([128, 6, 128], F32, tag="wst", name=f"wst{ci_t}{c}")
                    nc.sync.dma_start(st[:], w_d[ci_t, :, 6 * c:6 * c + 6, :])
                    nc.vector.tensor_copy(
                        w[:, ci_t * 36 + 6 * c: ci_t * 36 + 6 * c + 6, :], st[:])

            # per-(n,ci) strip state
            h3_t = [[None] * NS, [None] * NS]
            xb_t = [[None] * NSC, [None] * NSC]

            def load_x(n, ci, s):
                rs0, rs1 = R * s, min(R * s + R, HP)
                cnt = rs1 - rs0
                xt = xin_p.tile([128, cnt, WP], F32, tag="xin", name=f"x{n}{ci}{s}")
                # zero borders: cols [0,2) and [130,132); pad rows at ends
                nc.gpsimd.memset(xt[:, :, 0:2], 0.0)
                nc.gpsimd.memset(xt[:, :, WP - 2:WP], 0.0)
                xr0, xr1 = max(0, rs0 - 2), min(H, rs1 - 2)
                lr0, lr1 = xr0 - (rs0 - 2), xr1 - (rs0 - 2)
                if lr0 > 0:
                    nc.gpsimd.memset(xt[:, 0:lr0, 2:WP - 2], 0.0)
                if lr1 < cnt:
                    nc.gpsimd.memset(xt[:, lr1:cnt, 2:WP - 2], 0.0)
                nc.sync.dma_start(
                    xt[:, lr0:lr1, 2:WP - 2],
                    x_d[n, ci * 128:(ci + 1) * 128, xr0:xr1, :])
                return xt

            def h_chain(n, ci, s, xt):
                cnt = xt.shape[1]
                h1 = htmp_p.tile([128, R, WP - 1], F32, tag="h1", name=f"h1_{n}{ci}{s}")
                nc.gpsimd.tensor_add(h1[:, 0:cnt, :], xt[:, :, 0:WP - 1], xt[:, :, 1:WP])
                h2 = htmp_p.tile([128, R, WP - 2], F32, tag="h2", name=f"h2_{n}{ci}{s}")
                nc.gpsimd.tensor_add(h2[:, 0:cnt, :], h1[:, 0:cnt, 0:WP - 2],
                                     h1[:, 0:cnt, 1:WP - 1])
                h3 = h3_p.tile([128, R, WB], F32, tag=f"h3{ci}", name=f"h3_{n}{ci}{s}")
                nc.vector.tensor_add(h3[:, 0:cnt, :], h2[:, 0:cnt, 0:WB],
                                     h2[:, 0:cnt, 1:WB + 1])
                h3_t[ci][s] = (h3, cnt)

            MUL, ADD = mybir.AluOpType.mult, mybir.AluOpType.add

            def v_fused(n, ci, sg):
                """xb strip sg (0..NSC-1), rows [16sg, 16sg+17), via 4-tap STT
                chain on h3 rows: xb[r] = h3[r] + 3h3[r+1] + 3h3[r+2] + h3[r+3]
                = ((h3[r]/3 + h3[r+1]) + h3[r+2])*3 + h3[r+3]."""
                stt = nc.vector.scalar_tensor_tensor
                a, _ = h3_t[ci][sg]          # rows [16sg, 16sg+16)
                b, _ = h3_t[ci][sg + 1]      # next strip (>=4 rows)
                t1 = vt_p.tile([128, XBR + 1, WB], F32, tag="t1", name=f"t1_{n}{ci}{sg}")
                # t1[r] = h3[r]/3 + h3[r+1], r in [0, 18) local
                stt(t1[:, 0:15, :], a[:, 0:15, :], 1.0 / 3.0, a[:, 1:16, :], MUL, ADD)
                stt(t1[:, 15:16, :], a[:, 15:16, :], 1.0 / 3.0, b[:, 0:1, :], MUL, ADD)
                stt(t1[:, 16:18, :], b[:, 0:2, :], 1.0 / 3.0, b[:, 1:3, :], MUL, ADD)
                # t2[r] = t1[r] + h3[r+2], r in [0, 17)
                t2 = vt_p.tile([128, XBR, WB], F32, tag="t2", name=f"t2_{n}{ci}{sg}")
                nc.vector.tensor_add(t2[:, 0:14, :], t1[:, 0:14, :], a[:, 2:16, :])
                nc.vector.tensor_add(t2[:, 14:17, :], t1[:, 14:17, :], b[:, 0:3, :])
                # xb[r] = t2[r]*3 + h3[r+3], r in [0, 17), rounded to f32r
                t = xb_p.tile([128, XBR, WB], F32R, tag=f"xb{ci}", name=f"xb{n}{ci}{sg}")
                stt(t[:, 0:13, :], t2[:, 0:13, :], 3.0, a[:, 3:16, :], MUL, ADD)
                stt(t[:, 13:17, :], t2[:, 13:17, :], 3.0, b[:, 0:4, :], MUL, ADD)
                xb_t[ci][sg] = t

            def conv_strip(n, sp):
                for co_t in range(4):
                    pt = psum_p.tile([128, 8, WO], F32, tag="ps", name=f"ps{n}{sp}{co_t}")
                    k = 0
                    for ci in range(2):
                        xb = xb_t[ci][sp]
                        for u in range(3):
                            for v in range(3):
                                nc.tensor.matmul(
                                    pt[:],
                                    w[:, (ci * 9 + u * 3 + v) * 4 + co_t, :],
                                    xb[:, u:u + 15:2, v:v + 127:2],
                                    start=(k == 0), stop=(k == 17))
                                k += 1
                    o = out_p.tile([128, 8, WO], F32, tag="o", name=f"o{n}{sp}{co_t}")
                    nc.scalar.activation(o[:], pt[:], AF.Identity,
                                         bias=bias[:, co_t:co_t + 1], scale=1.0)
                    nc.sync.dma_start(
                        y_d[n, co_t * 128:(co_t + 1) * 128, 8 * sp:8 * sp + 8, :],
                        o[:])

            for n in range(N_PC):
                for s in range(NS + 1):
                    for ci in range(2):
                        if s < NS:
                            xt = load_x(n, ci, s)
                            h_chain(n, ci, s, xt)
                        if 1 <= s and s - 1 < NSC:
                            v_fused(n, ci, s - 1)
                    if 1 <= s and s - 1 < NSC:
                        conv_strip(n, s - 1)
    return nc


# ------------------------------------------------------------- PJRT runner
class _Runner:
    def __init__(self, nc, n_cores):
        import jax
        import concourse.mybir as mybir
        from jax.sharding import Mesh, PartitionSpec
        from jax.experimental.shard_map import shard_map
        from concourse.bass2jax import (
            _bass_exec_p, install_neuronx_cc_hook, partition_id_tensor)

        install_neuronx_cc_hook()
        self.jax = jax
        self.n_cores = n_cores
        pname = nc.partition_id_tensor.name if nc.partition_id_tensor else None
        in_names, out_names, out_avals = [], [], []
        for alloc in nc.m.functions[0].allocations:
            if not isinstance(alloc, mybir.MemoryLocationSet):
                continue
            name = alloc.memorylocations[0].name
            if alloc.kind == "ExternalInput":
                if name != pname:
                    in_names.append(name)
            elif alloc.kind == "ExternalOutput":
                out_names.append(name)
                out_avals.append(jax.core.ShapedArray(
                    tuple(alloc.tensor_shape), mybir.dt.np(alloc.dtype)))
        self.in_names, self.out_names, self.out_avals = in_names, out_names, out_avals
        n_params, n_outs = len(in_names), len(out_names)
        self.n_params = n_params
        all_in = list(in_names) + list(out_names)
        if pname is not None:
            all_in.append(pname)
        donate = tuple(range(n_params, n_params + n_outs))

        def _body(*args):
            operands = list(args)
            if pname is not None:
                operands.append(partition_id_tensor())
            return tuple(_bass_exec_p.bind(
                *operands, out_avals=tuple(out_avals), in_names=tuple(all_in),
                out_names=tuple(out_names), lowering_input_output_aliases=(),
                sim_require_finite=False, sim_require_nnan=False, nc=nc))

        devices = jax.devices()[:n_cores]
        mesh = Mesh(np.asarray(devices), ("core",))
        self.fn = jax.jit(
            shard_map(_body, mesh=mesh,
                      in_specs=(PartitionSpec("core"),) * (n_params + n_outs),
                      out_specs=(PartitionSpec("core"),) * n_outs,
                      check_rep=False),
            donate_argnums=donate, keep_unused=True)

    def run(self, concat_inputs):
        zeros = [np.zeros((self.n_cores * a.shape[0], *a.shape[1:]), a.dtype)
                 for a in self.out_avals]
        outs = self.fn(*concat_inputs, *zeros)
        self.jax.block_until_ready(outs)
        return [np.asarray(o) for o in outs]


def _get_runner():
    if "runner" not in _CACHE:
        _install_birfix()
        nc = _build_module()
        _CACHE["runner"] = _Runner(nc, N_CORES)
    return _CACHE["runner"]


# ------------------------------------------------------------------ kernel
def kernel(x, weight, bias, blur_k):
    x = np.asarray(x, dtype=np.float32)
    weight = np.asarray(weight, dtype=np.float32)
    bias_np = np.asarray(bias, dtype=np.float32)

    scale = 1.0 / np.sqrt(weight.shape[1] * weight.shape[2] * weight.shape[3])
    weff = weight * np.float32(scale / 64.0)
    # lhsT layout [ci_t, ci, tap*4+co_t, co]
    a = weff.transpose(1, 2, 3, 0)              # [256ci, 3u, 3v, 512co]
    a = a.reshape(2, 128, 9, 4, 128)            # [ci_t, ci, tap, co_t, co]
    wl = np.ascontiguousarray(a.reshape(2, 128, 36, 128), dtype=np.float32)
    br = np.ascontiguousarray(bias_np.reshape(4, 128).T, dtype=np.float32)  # [128,4]

    r = _get_runner()
    shards = x.reshape(N_CORES, N_PC, C_IN, H, W)
    concat = []
    for name in r.in_names:
        if name == "x":
            concat.append(shards.reshape(N_CORES * N_PC, C_IN, H, W))
        elif name == "w":
            concat.append(np.concatenate([wl] * N_CORES, axis=0))
        elif name == "b":
            concat.append(np.concatenate([br] * N_CORES, axis=0))
    outs = r.run([np.ascontiguousarray(c) for c in concat])
    y = outs[r.out_names.index("y")]
    return np.ascontiguousarray(y.reshape(N_FULL, C_OUT, HO, WO))
